# revision 31
# baseline (speedup 1.0000x reference)
"""Banded (sparse) attention encoder block on 8 Trainium2 NeuronCores.

Problem: nn_NeuralEncoder (B=4, S=2000=100 time patches x 20 space patches,
H=512, 8 heads, D=64, RoPE over time-patch timestamps, banded attention
|t_q - t_k| <= 4 tiled over space patches).

Sharding: 8 cores = 4 batches x 2 head-groups (4 heads each).
Host prep: permute tokens to time-major order (u = t*SP + sp) so the banded
mask becomes a contiguous band of keys; transpose x to xT [H, S]; slice
weights per head-group; build small RoPE tables + band-bias matrices.

Device schedule (v2) is built to keep the PE (tensor engine) gap-free so it
ramps to and holds its top p-state:
  - q/k projections + RoPE pipelined: 4-matmul accum chains feed an ACT
    psum->sbuf copy; the rotate-half PE matmul trails 2 chains behind so it
    never waits; v-projection chains fill the gaps.
  - band mask applied INSIDE the scores psum via a rank-6 bias matmul
    (one-hot key-patch matrix x {0,-3000} pattern), so exp of out-of-band
    slots is exactly 0 and no vector mask multiply exists.
  - attn_mask applied via the ACT exp per-partition bias port (-3000 * (1-m)).
  - softmax normalize = one DVE reciprocal + ONE broadcast tensor_tensor for
    all 4 heads.
  - AV accumulates per 120-query strip in natural [q, head, d+1] layout with
    the denominator in column 64 (ones column in v).
  - context transposed to [hd, s] via PE transposes; Wo matmul interleaved
    into the attention tail (after strips 4/8/12/16) to fill the PE.
  - all PSUM draws from a single 6-slot ring (+2 transpose slots) = 8 banks.
Host epilogue: sum the two head-group partials per batch, add bo, transpose,
un-permute back to the original space-major token order.
"""

import numpy as np
import ml_dtypes
from contextlib import ExitStack

import concourse.tile as tile
from concourse import bacc, mybir
from concourse import bass_utils

F32 = mybir.dt.float32
BF16 = mybir.dt.bfloat16

# Static problem configuration (hardcoded, matches the reference).
B, T, SP = 4, 100, 20
S = T * SP                  # 2000
H, NH, D = 512, 8, 64
CF = CB = 4
G = 2                       # head groups (tensor-parallel factor)
HPC = NH // G               # heads per core = 4
HG = HPC * D                # 256 hidden per group
ROPE_BASE = 10000.0
N_CORES = 8
NEGB = -3000.0              # band/mask bias (exp(-3000*0.125) == 0 in f32)

PPC = 6                     # time patches per key chunk
NCH = (T + PPC - 1) // PPC  # 17 chunks / query strips
SC = 500                    # free-dim chunk for [128, 500] proj tiles
NSC = S // SC               # 4
MW = (2 * PPC + CF) * SP    # 320: max scoresT query-window width
# windows start at the previous strip boundary (6(j-1)) so every AV psum
# write begins at partition row 0 (PE requires out base partition 0/32/64)

_CACHE = {}


def _pat(j):
    return 6 * j, min(T, 6 * j + PPC)


def _ck(j):
    p0, p1 = _pat(j)
    return (p1 - p0) * SP


def _qwin(j):
    """Token range of the query window covered by scoresT of key chunk j."""
    p0, p1 = _pat(j)
    return max(0, p0 - PPC) * SP, min(T, p1 + CF) * SP


def _build_program():
    nc = bacc.Bacc("TRN2", target_bir_lowering=False, debug=False,
                   num_devices=N_CORES)

    xT = nc.dram_tensor("xT", [H, S], BF16, kind="ExternalInput").ap()
    wq = nc.dram_tensor("wq", [H, HG], BF16, kind="ExternalInput").ap()
    wk = nc.dram_tensor("wk", [H, HG], BF16, kind="ExternalInput").ap()
    wv = nc.dram_tensor("wv", [H, HG], BF16, kind="ExternalInput").ap()
    wo = nc.dram_tensor("wo", [HG, H], BF16, kind="ExternalInput").ap()
    cosP = nc.dram_tensor("cosP", [128, T], BF16, kind="ExternalInput").ap()
    sinP = nc.dram_tensor("sinP", [128, T], BF16, kind="ExternalInput").ap()
    p128 = nc.dram_tensor("p128", [128, 128], BF16, kind="ExternalInput").ap()
    idt = nc.dram_tensor("idt", [120, 120], BF16, kind="ExternalInput").ap()
    m_first = nc.dram_tensor("m_first", [120, 200], BF16,
                             kind="ExternalInput").ap()
    m_int = nc.dram_tensor("m_int", [120, MW - 40], BF16,
                           kind="ExternalInput").ap()
    amb = nc.dram_tensor("amb", [120, NCH], F32, kind="ExternalInput").ap()
    outT = nc.dram_tensor("outT", [H, S], F32, kind="ExternalOutput").ap()
    import os
    dbg = os.environ.get("KDBG", "") == "1"
    if dbg:
        qTo = nc.dram_tensor("qTo", [2, 128, S], BF16, kind="ExternalOutput").ap()
        kTo = nc.dram_tensor("kTo", [2, 128, S], BF16, kind="ExternalOutput").ap()
        cxo = nc.dram_tensor("cxo", [2, 128, S], BF16, kind="ExternalOutput").ap()
        vo = nc.dram_tensor("vo", [120, NCH, HPC, D + 1], BF16,
                            kind="ExternalOutput").ap()
        cso = nc.dram_tensor("cso", [NCH, 120, HPC, D], BF16,
                             kind="ExternalOutput").ap()

    with ExitStack() as ctx:
        tc = ctx.enter_context(tile.TileContext(nc))
        consts = ctx.enter_context(tc.tile_pool(name="consts", bufs=1))
        persist = ctx.enter_context(tc.tile_pool(name="persist", bufs=1))
        work = ctx.enter_context(tc.tile_pool(name="work", bufs=3))
        pu = ctx.enter_context(tc.tile_pool(name="pu", bufs=1, space="PSUM"))
        pt = ctx.enter_context(tc.tile_pool(name="pt", bufs=1, space="PSUM"))

        def utile():
            return pu.tile([128, 512], F32, tag="u", name="u", bufs=3)

        def u2tile():
            return pu.tile([128, 2, 512], F32, tag="u2", name="u2", bufs=1)

        def uvtile():
            return pu.tile([128, 512], F32, tag="uv", name="uv", bufs=1)

        def pavtile():
            return pu.tile([128, 512], F32, tag="pav", name="pav", bufs=1)

        # ---- constants into SBUF (DMA priority order: first-needed first) --
        p_sb = consts.tile([128, 128], BF16, tag="p128")
        nc.sync.dma_start(out=p_sb, in_=p128)
        wq_sb = consts.tile([128, 4, HG], BF16, tag="wq")
        nc.sync.dma_start(out=wq_sb, in_=wq.rearrange("(c p) m -> p c m", p=128))
        wk_sb = consts.tile([128, 4, HG], BF16, tag="wk")
        nc.sync.dma_start(out=wk_sb, in_=wk.rearrange("(c p) m -> p c m", p=128))
        # x in [128, 500] slices so the first projection can start early
        xt = [consts.tile([128, S], BF16, tag=f"xt{kc}", name=f"xt{kc}")
              for kc in range(4)]
        for sc in range(NSC):
            cols = slice(SC * sc, SC * (sc + 1))
            for kc in range(4):
                nc.sync.dma_start(out=xt[kc][:, cols],
                                  in_=xT[128 * kc:128 * (kc + 1), cols])
            if sc == 0:
                cosP_sb = consts.tile([128, T], BF16, tag="cosP")
                nc.sync.dma_start(out=cosP_sb, in_=cosP)
                sinP_sb = consts.tile([128, T], BF16, tag="sinP")
                nc.sync.dma_start(out=sinP_sb, in_=sinP)
                wv_sb = consts.tile([128, 4, HG], BF16, tag="wv")
                nc.sync.dma_start(out=wv_sb,
                                  in_=wv.rearrange("(c p) m -> p c m", p=128))
        mf_sb = consts.tile([120, 200], BF16, tag="mf")
        nc.sync.dma_start(out=mf_sb, in_=m_first)
        mi_sb = consts.tile([120, MW - 40], BF16, tag="mi")
        nc.sync.dma_start(out=mi_sb, in_=m_int)
        amb_sb = consts.tile([120, NCH], F32, tag="amb")
        nc.sync.dma_start(out=amb_sb, in_=amb)
        id_sb = consts.tile([120, 120], BF16, tag="idt")
        nc.sync.dma_start(out=id_sb, in_=idt)
        wo_sb = consts.tile([128, 2, H], BF16, tag="wo")
        nc.sync.dma_start(out=wo_sb, in_=wo.rearrange("(c p) m -> p c m", p=128))

        # expand RoPE tables [128, T] -> [128, T, SP] on-chip (DVE)
        cos_e = consts.tile([128, T, SP], BF16, tag="cos")
        nc.vector.tensor_copy(
            out=cos_e, in_=cosP_sb[:, :, None].broadcast_to([128, T, SP]))
        sin_e = consts.tile([128, T, SP], BF16, tag="sin")
        nc.vector.tensor_copy(
            out=sin_e, in_=sinP_sb[:, :, None].broadcast_to([128, T, SP]))
        cos_sb = cos_e.rearrange("p t s -> p (t s)")
        sin_sb = sin_e.rearrange("p t s -> p (t s)")

        # ---- persistent activations ----
        qT = [persist.tile([128, S], BF16, tag=f"qT{hp}", name=f"qT{hp}")
              for hp in range(2)]
        kT = [persist.tile([128, S], BF16, tag=f"kT{hp}", name=f"kT{hp}")
              for hp in range(2)]
        ctxT = [persist.tile([128, S], BF16, tag=f"ctxT{hp}", name=f"ctxT{hp}")
                for hp in range(2)]
        # v in natural layout per chunk: [key, head, d+1]; col 64 = denom ones
        v_sb = persist.tile([120, NCH, HPC, D + 1], BF16, tag="v", name="v_sb")
        nc.gpsimd.memset(v_sb[:, :, :, D], 1.0)
        # exp ring: 12 pair-slots (6 chunks deep); frame cols [0:40) are the
        # never-written left pad -> zeroed ONCE so AV contraction skips them
        ERING = 12
        exps = persist.tile([120, ERING, 2, MW], BF16, tag="exps",
                            name="exps")
        nc.gpsimd.memset(exps[:, :, :, 0:40], 0.0)
        exp0 = persist.tile([120, 2, 2, 200], BF16, tag="exp0", name="exp0")

        # ---- merged projection + attention schedule ----
        # qk/v units interleave with attention chunk-steps so the PE stream
        # stays dense (HAM clock-gate stays released at 2.4 GHz)
        units = []
        qk_seq = [(w, hp, sc) for sc in range(NSC)
                  for (w, hp) in (("q", 0), ("k", 0), ("q", 1), ("k", 1))]
        for i, u in enumerate(qk_seq):
            units.append(("qk",) + u)
            if i < NCH:
                units.append(("v", i))
        units.append(("v", 16))
        ucur = [0]
        state = {"tokens_ready": 0, "v_emitted": 0}

        def emit_v(vc, ps):
            ckv = _ck(vc)
            rows = slice(120 * vc, 120 * vc + ckv)
            for kc in range(4):
                nc.tensor.matmul(
                    ps[0:ckv, :],
                    lhsT=xt[kc][:, rows],
                    rhs=wv_sb[:, kc, :],
                    start=(kc == 0), stop=(kc == 3),
                )
            vdst = v_sb[0:ckv, vc, :, 0:D]
            vsrc = ps[0:ckv, 0:HG].rearrange("p (h e) -> p h e", e=D)
            if vc % 2 == 0:
                nc.scalar.copy(out=vdst, in_=vsrc)
            else:
                nc.vector.tensor_copy(out=vdst, in_=vsrc)
            state["v_emitted"] += 1

        # PE warm-up: junk matmuls on p128 while the first DMAs land
        wps = utile()
        for r in range(80):
            nc.tensor.matmul(wps[:, 0:128], lhsT=p_sb, rhs=p_sb,
                             start=True, stop=True)
        wdump = work.tile([1, 1], F32, tag="wdump", bufs=1)
        nc.vector.tensor_copy(out=wdump, in_=wps[0:1, 0:1])

        rot_q = []          # pending rotate-half jobs: (w, hp, sc, ps, pre)
        rots_done = {sc: 0 for sc in range(NSC)}

        def emit_rot():
            w, hp, sc, ps, pre = rot_q.pop(0)
            cols = slice(SC * sc, SC * (sc + 1))
            dst = (qT if w == "q" else kT)[hp]
            nc.tensor.matmul(ps, lhsT=p_sb, rhs=pre, start=True, stop=True)
            t1 = work.tile([128, SC], BF16, tag="t1")
            nc.vector.tensor_mul(out=t1, in0=pre, in1=cos_sb[:, cols])
            t2 = work.tile([128, SC], BF16, tag="t2")
            nc.vector.tensor_mul(out=t2, in0=ps, in1=sin_sb[:, cols])
            nc.gpsimd.tensor_add(out=dst[:, cols], in0=t1, in1=t2)
            rots_done[sc] += 1
            if rots_done[sc] == 4:
                state["tokens_ready"] = SC * (sc + 1)

        def emit_unit():
            if ucur[0] >= len(units):
                while rot_q:
                    emit_rot()
                return False
            unit = units[ucur[0]]
            ucur[0] += 1
            if unit[0] == "qk":
                _, w, hp, sc = unit
                cols = slice(SC * sc, SC * (sc + 1))
                w_sb = wq_sb if w == "q" else wk_sb
                ps = utile()[:, 0:SC]
                for kc in range(4):
                    nc.tensor.matmul(
                        ps,
                        lhsT=w_sb[:, kc, 128 * hp:128 * (hp + 1)],
                        rhs=xt[kc][:, cols],
                        start=(kc == 0), stop=(kc == 3),
                    )
                pre = work.tile([128, SC], BF16, tag="pre", bufs=4)
                nc.scalar.copy(out=pre, in_=ps)
                rot_q.append((w, hp, sc, ps, pre))
            else:
                emit_v(unit[1], uvtile()[:, 0:HG])
            if len(rot_q) >= 2:
                emit_rot()
            return True

        # ---- attention ----
        exp_t = {}      # (j, h) -> (exp AP [keys, MW-ish], frame base token)

        def scores_chunk(j, p):
            qlo, qhi = _qwin(j)
            ckj = _ck(j)
            cj = slice(120 * j, 120 * j + ckj)
            fb = 0 if j == 0 else 120 * (j - 1)   # frame base token
            c0 = 0 if j == 0 else 40              # first written frame col
            w = qhi - fb
            ps = u2tile()
            for e in range(2):
                h = 2 * p + e
                hp, hb = h // 2, 64 * (h % 2)
                nc.tensor.matmul(
                    ps[0:ckj, e, c0:w],
                    lhsT=kT[hp][hb:hb + 64, cj],
                    rhs=qT[hp][hb:hb + 64, fb + c0:qhi],
                    start=True, stop=True,
                )
            et = exp0[:, p] if j == 0 else exps[:, (2 * j + p) % ERING]
            nc.scalar.activation(out=et[0:ckj, :, c0:w],
                                 in_=ps[0:ckj, :, c0:w],
                                 func=mybir.ActivationFunctionType.Exp,
                                 scale=0.125,
                                 bias=amb_sb[0:ckj, j:j + 1])
            mk = mf_sb[0:ckj, 0:w] if j == 0 else mi_sb[0:ckj, 0:w - 40]
            nc.vector.tensor_mul(
                out=et[0:ckj, :, c0:w], in0=et[0:ckj, :, c0:w],
                in1=mk[:, None, :].broadcast_to([ckj, 2, w - c0]))
            for e in range(2):
                exp_t[(j, 2 * p + e)] = (et[:, e], fb)

        pend_tr = []

        def pop_tr(n=1):
            for _ in range(n):
                if not pend_tr:
                    return
                i, hp = pend_tr.pop(0)
                cki = _ck(i)
                cs = tr_cs[i]
                csf = cs.rearrange("p h e -> p (h e)")
                ptr = pt.tile([128, 120], BF16, tag="t", name="ptr")
                nc.tensor.transpose(ptr[:, 0:cki],
                                    csf[0:cki, 128 * hp:128 * (hp + 1)],
                                    id_sb[0:cki, 0:cki])
                nc.vector.tensor_copy(
                    out=ctxT[hp][:, 120 * i:120 * i + cki], in_=ptr[:, 0:cki])

        tr_cs = {}

        def av_strip(i):
            # chunk i first: it covers the strip fully (start=True sets
            # has_written; neighbors accumulate on partial row ranges)
            cki = _ck(i)
            chunks = [c for c in (i, i - 1, i + 1) if 0 <= c < NCH]
            psu = pavtile()
            ps = psu[:, 0:HPC * (D + 1)].rearrange("p (h e) -> p h e", e=D + 1)
            for h in range(HPC):
                for n, jc in enumerate(chunks):
                    qlo, qhi = _qwin(jc)
                    lo_g = max(120 * i, qlo)
                    hi_g = min(120 * i + cki, qhi)
                    assert lo_g == 120 * i
                    ev, fb = exp_t[(jc, h)]
                    nc.tensor.matmul(
                        ps[0:hi_g - lo_g, h, :],
                        lhsT=ev[0:_ck(jc), lo_g - fb:hi_g - fb],
                        rhs=v_sb[0:_ck(jc), jc, h, :],
                        start=(n == 0), stop=(n == len(chunks) - 1),
                    )
            # softmax normalize: rcp of denom col, ONE broadcast multiply
            rcp = work.tile([120, HPC], F32, tag="rcp")
            nc.vector.reciprocal(out=rcp[0:cki, :], in_=ps[0:cki, :, D])
            cs = work.tile([120, HPC, D], BF16, tag="cs", bufs=4)
            nc.vector.tensor_mul(
                out=cs[0:cki], in0=ps[0:cki, :, 0:D],
                in1=rcp[0:cki, :, None].broadcast_to([cki, HPC, D]))
            if dbg:
                nc.sync.dma_start(out=cso[i, 0:cki], in_=cs[0:cki])
            tr_cs[i] = cs
            pend_tr.append((i, 0))
            pend_tr.append((i, 1))

        def out_proj(lo, hi):
            for oc in range(4):
                ps = utile()[:, 0:hi - lo]
                for hp in range(2):
                    nc.tensor.matmul(
                        ps,
                        lhsT=wo_sb[:, hp, 128 * oc:128 * (oc + 1)],
                        rhs=ctxT[hp][:, lo:hi],
                        start=(hp == 0), stop=(hp == 1),
                    )
                ost = work.tile([128, SC], F32, tag="ost")
                if oc % 2 == 0:
                    nc.scalar.copy(out=ost[:, 0:hi - lo], in_=ps)
                else:
                    nc.vector.tensor_copy(out=ost[:, 0:hi - lo], in_=ps)
                nc.sync.dma_start(out=outT[128 * oc:128 * (oc + 1), lo:hi],
                                  in_=ost[:, 0:hi - lo])

        for j in range(NCH):
            while (state["tokens_ready"] < min(S, _qwin(j)[1] + 400)
                   or state["v_emitted"] < min(NCH, j + 2)):
                if not emit_unit():
                    break
            scores_chunk(j, 0)
            emit_unit()
            pop_tr()
            scores_chunk(j, 1)
            emit_unit()
            pop_tr()
            if j >= 3:
                av_strip(j - 3)
                emit_unit()
                if j - 3 == 9:
                    out_proj(0, 500)
                    out_proj(500, 1000)
                elif j - 3 == 13:
                    out_proj(1000, 1500)
        while emit_unit():
            pass
        av_strip(NCH - 3)
        pop_tr()
        av_strip(NCH - 2)
        pop_tr(3)
        out_proj(1500, 1800)
        av_strip(NCH - 1)
        pop_tr(4)
        out_proj(1800, 2000)
        if dbg:
            for hp in range(2):
                nc.sync.dma_start(out=qTo[hp], in_=qT[hp])
                nc.sync.dma_start(out=kTo[hp], in_=kT[hp])
                nc.sync.dma_start(out=cxo[hp], in_=ctxT[hp])
            nc.sync.dma_start(out=vo, in_=v_sb)

    nc.finalize()   # Bacc register allocation + DCE before serialization
    return nc


def _get_program():
    if "nc" not in _CACHE:
        _CACHE["nc"] = _build_program()
    return _CACHE["nc"]


def _host_prep(x, attn_mask, timestamps, Wq, Wk, Wv, Wo):
    """Build the 8 per-core input maps."""
    bf16 = ml_dtypes.bfloat16

    def to_tm(a):
        # [B, S, ...] space-major -> time-major (u = t*SP + sp)
        return (a.reshape(B, SP, T, *a.shape[2:])
                 .swapaxes(1, 2)
                 .reshape(B, S, *a.shape[2:]))

    x_tm = to_tm(np.ascontiguousarray(x))
    amask_tm = to_tm(np.ascontiguousarray(attn_mask)).astype(np.float32)

    # RoPE tables (per time patch; expanded to tokens on-chip)
    inv_freq = 1.0 / (ROPE_BASE ** (np.arange(0, D, 2, dtype=np.float32) / D))
    tt = np.arange(T, dtype=np.float32)
    freqs = tt[:, None] * inv_freq[None, :]
    emb = np.concatenate([freqs, freqs], axis=-1)      # [T, D]
    cos_t = np.cos(emb).astype(np.float32).T           # [64, T]
    sin_t = np.sin(emb).astype(np.float32).T
    cosP = np.vstack([cos_t, cos_t])                   # [128, T]
    sinP = np.vstack([sin_t, sin_t])

    # rotation matrix (sign-carrying rotate-half), block-diag per head pair
    p = np.zeros((128, 128), np.float32)
    for blk in (0, 64):
        for d in range(32):
            p[blk + d + 32, blk + d] = -1.0
            p[blk + d, blk + d + 32] = 1.0

    # band masks: interior chunk j has frame at patch 6(j-1); key k (patch
    # r=k//20) sees query rel-patch pc iff pc - r in [2, 10]; the exp tile
    # stores cols [40:w] so m_int covers frame cols 40..320
    rr = np.arange(120)[:, None] // SP
    pc_i = (np.arange(MW - 40)[None, :] + 40) // SP
    m_int = ((pc_i - rr >= 2) & (pc_i - rr <= 10)).astype(np.float32)
    pc_f = np.arange(200)[None, :] // SP
    m_first = ((pc_f - rr >= -4) & (pc_f - rr <= 4)).astype(np.float32)

    # attn_mask bias per (key-in-chunk, chunk)
    def amb_of(am_b):
        out = np.zeros((120, NCH), np.float32)
        for j in range(NCH):
            ckj = _ck(j)
            out[0:ckj, j] = NEGB * (1.0 - am_b[120 * j:120 * j + ckj])
        return out

    in_maps = []
    for c in range(N_CORES):
        b, g = c // 2, c % 2
        hcols = slice(HG * g, HG * (g + 1))
        in_maps.append({
            "xT": np.ascontiguousarray(x_tm[b].T).astype(bf16),
            "wq": np.ascontiguousarray(Wq[:, hcols]).astype(bf16),
            "wk": np.ascontiguousarray(Wk[:, hcols]).astype(bf16),
            "wv": np.ascontiguousarray(Wv[:, hcols]).astype(bf16),
            "wo": np.ascontiguousarray(Wo[hcols, :]).astype(bf16),
            "cosP": cosP.astype(bf16),
            "sinP": sinP.astype(bf16),
            "p128": p.astype(bf16),
            "idt": np.eye(120, dtype=np.float32).astype(bf16),
            "m_first": m_first.astype(bf16),
            "m_int": m_int.astype(bf16),
            "amb": amb_of(amask_tm[b]),
        })
    return in_maps


def kernel(x, attn_mask, timestamps, Wq, bq, Wk, bk, Wv, bv, Wo, bo,
           **_ignored):
    x = np.asarray(x, np.float32)
    attn_mask = np.asarray(attn_mask)
    timestamps = np.asarray(timestamps)
    Wq, Wk, Wv, Wo = (np.asarray(a, np.float32) for a in (Wq, Wk, Wv, Wo))
    bq, bk, bv, bo = (np.asarray(a, np.float32) for a in (bq, bk, bv, bo))
    assert not (np.any(bq) or np.any(bk) or np.any(bv)), \
        "nonzero qkv biases not supported"

    nc = _get_program()
    in_maps = _host_prep(x, attn_mask, timestamps, Wq, Wk, Wv, Wo)

    res = bass_utils.run_bass_kernel_spmd(nc, in_maps,
                                          core_ids=list(range(N_CORES)))
    _CACHE["last_results"] = res

    out = np.empty((B, S, H), np.float32)
    for b in range(B):
        o = res.results[2 * b]["outT"] + res.results[2 * b + 1]["outT"]
        o_tm = o.T + bo[None, :]                        # [2000, 512]
        out[b] = (o_tm.reshape(T, SP, H)
                      .swapaxes(0, 1)
                      .reshape(S, H))
    return out


# revision 32
# speedup vs baseline: 1.2052x; 1.2052x over previous
"""Banded (sparse) attention encoder block on 8 Trainium2 NeuronCores.

Problem: nn_NeuralEncoder (B=4, S=2000=100 time patches x 20 space patches,
H=512, 8 heads, D=64, RoPE over time-patch timestamps, banded attention
|t_q - t_k| <= 4 tiled over space patches).

Sharding: 8 cores = 4 batches x 2 head-groups (4 heads each).
Host prep: permute tokens to time-major order (u = t*SP + sp) so the banded
mask becomes a contiguous band of keys; transpose x to xT [H, S]; slice
weights per head-group; build small RoPE tables + band-bias matrices.

Device schedule (v2) is built to keep the PE (tensor engine) gap-free so it
ramps to and holds its top p-state:
  - q/k projections + RoPE pipelined: 4-matmul accum chains feed an ACT
    psum->sbuf copy; the rotate-half PE matmul trails 2 chains behind so it
    never waits; v-projection chains fill the gaps.
  - band mask applied INSIDE the scores psum via a rank-6 bias matmul
    (one-hot key-patch matrix x {0,-3000} pattern), so exp of out-of-band
    slots is exactly 0 and no vector mask multiply exists.
  - attn_mask applied via the ACT exp per-partition bias port (-3000 * (1-m)).
  - softmax normalize = one DVE reciprocal + ONE broadcast tensor_tensor for
    all 4 heads.
  - AV accumulates per 120-query strip in natural [q, head, d+1] layout with
    the denominator in column 64 (ones column in v).
  - context transposed to [hd, s] via PE transposes; Wo matmul interleaved
    into the attention tail (after strips 4/8/12/16) to fill the PE.
  - all PSUM draws from a single 6-slot ring (+2 transpose slots) = 8 banks.
Host epilogue: sum the two head-group partials per batch, add bo, transpose,
un-permute back to the original space-major token order.
"""

import numpy as np
import ml_dtypes
from contextlib import ExitStack

import concourse.tile as tile
from concourse import bacc, mybir
from concourse import bass_utils

F32 = mybir.dt.float32
BF16 = mybir.dt.bfloat16

# Static problem configuration (hardcoded, matches the reference).
B, T, SP = 4, 100, 20
S = T * SP                  # 2000
H, NH, D = 512, 8, 64
CF = CB = 4
G = 2                       # head groups (tensor-parallel factor)
HPC = NH // G               # heads per core = 4
HG = HPC * D                # 256 hidden per group
ROPE_BASE = 10000.0
N_CORES = 8
NEGB = -3000.0              # band/mask bias (exp(-3000*0.125) == 0 in f32)

PPC = 6                     # time patches per key chunk
NCH = (T + PPC - 1) // PPC  # 17 chunks / query strips
SC = 500                    # free-dim chunk for [128, 500] proj tiles
NSC = S // SC               # 4
MW = (2 * PPC + CF) * SP    # 320: max scoresT query-window width
# windows start at the previous strip boundary (6(j-1)) so every AV psum
# write begins at partition row 0 (PE requires out base partition 0/32/64)

_CACHE = {}


def _pat(j):
    return 6 * j, min(T, 6 * j + PPC)


def _ck(j):
    p0, p1 = _pat(j)
    return (p1 - p0) * SP


def _qwin(j):
    """Token range of the query window covered by scoresT of key chunk j."""
    p0, p1 = _pat(j)
    return max(0, p0 - PPC) * SP, min(T, p1 + CF) * SP


def _build_program():
    nc = bacc.Bacc("TRN2", target_bir_lowering=False, debug=False,
                   num_devices=N_CORES)

    xT = nc.dram_tensor("xT", [H, S], BF16, kind="ExternalInput").ap()
    wq = nc.dram_tensor("wq", [H, HG], BF16, kind="ExternalInput").ap()
    wk = nc.dram_tensor("wk", [H, HG], BF16, kind="ExternalInput").ap()
    wv = nc.dram_tensor("wv", [H, HG], BF16, kind="ExternalInput").ap()
    wo = nc.dram_tensor("wo", [HG, H], BF16, kind="ExternalInput").ap()
    cosP = nc.dram_tensor("cosP", [128, T], BF16, kind="ExternalInput").ap()
    sinP = nc.dram_tensor("sinP", [128, T], BF16, kind="ExternalInput").ap()
    p128 = nc.dram_tensor("p128", [128, 128], BF16, kind="ExternalInput").ap()
    idt = nc.dram_tensor("idt", [120, 120], BF16, kind="ExternalInput").ap()
    m_first = nc.dram_tensor("m_first", [120, 200], BF16,
                             kind="ExternalInput").ap()
    m_int = nc.dram_tensor("m_int", [120, MW - 40], BF16,
                           kind="ExternalInput").ap()
    amb = nc.dram_tensor("amb", [120, NCH], F32, kind="ExternalInput").ap()
    outT = nc.dram_tensor("outT", [H, S], F32, kind="ExternalOutput").ap()
    import os
    dbg = os.environ.get("KDBG", "") == "1"
    if dbg:
        qTo = nc.dram_tensor("qTo", [2, 128, S], BF16, kind="ExternalOutput").ap()
        kTo = nc.dram_tensor("kTo", [2, 128, S], BF16, kind="ExternalOutput").ap()
        cxo = nc.dram_tensor("cxo", [2, 128, S], BF16, kind="ExternalOutput").ap()
        vo = nc.dram_tensor("vo", [120, NCH, HPC, D + 1], BF16,
                            kind="ExternalOutput").ap()
        cso = nc.dram_tensor("cso", [NCH, 120, HPC, D], BF16,
                             kind="ExternalOutput").ap()

    with ExitStack() as ctx:
        tc = ctx.enter_context(tile.TileContext(nc))
        consts = ctx.enter_context(tc.tile_pool(name="consts", bufs=1))
        persist = ctx.enter_context(tc.tile_pool(name="persist", bufs=1))
        work = ctx.enter_context(tc.tile_pool(name="work", bufs=3))
        pu = ctx.enter_context(tc.tile_pool(name="pu", bufs=1, space="PSUM"))
        pt = ctx.enter_context(tc.tile_pool(name="pt", bufs=1, space="PSUM"))

        def utile():
            return pu.tile([128, 512], F32, tag="u", name="u", bufs=3)

        def u2tile():
            return pu.tile([128, 2, 512], F32, tag="u2", name="u2", bufs=1)

        def uvtile():
            return pu.tile([128, 512], F32, tag="uv", name="uv", bufs=1)

        def pavtile():
            return pu.tile([128, 512], F32, tag="pav", name="pav", bufs=1)

        # ---- constants into SBUF (DMA priority order: first-needed first) --
        p_sb = consts.tile([128, 128], BF16, tag="p128")
        nc.sync.dma_start(out=p_sb, in_=p128)
        wq_sb = consts.tile([128, 4, HG], BF16, tag="wq")
        nc.sync.dma_start(out=wq_sb, in_=wq.rearrange("(c p) m -> p c m", p=128))
        wk_sb = consts.tile([128, 4, HG], BF16, tag="wk")
        nc.sync.dma_start(out=wk_sb, in_=wk.rearrange("(c p) m -> p c m", p=128))
        # x in [128, 500] slices so the first projection can start early
        xt = [consts.tile([128, S], BF16, tag=f"xt{kc}", name=f"xt{kc}")
              for kc in range(4)]
        for sc in range(NSC):
            cols = slice(SC * sc, SC * (sc + 1))
            for kc in range(4):
                nc.sync.dma_start(out=xt[kc][:, cols],
                                  in_=xT[128 * kc:128 * (kc + 1), cols])
            if sc == 0:
                cosP_sb = consts.tile([128, T], BF16, tag="cosP")
                nc.sync.dma_start(out=cosP_sb, in_=cosP)
                sinP_sb = consts.tile([128, T], BF16, tag="sinP")
                nc.sync.dma_start(out=sinP_sb, in_=sinP)
                wv_sb = consts.tile([128, 4, HG], BF16, tag="wv")
                nc.sync.dma_start(out=wv_sb,
                                  in_=wv.rearrange("(c p) m -> p c m", p=128))
        mf_sb = consts.tile([120, 200], BF16, tag="mf")
        nc.sync.dma_start(out=mf_sb, in_=m_first)
        mi_sb = consts.tile([120, MW - 40], BF16, tag="mi")
        nc.sync.dma_start(out=mi_sb, in_=m_int)
        amb_sb = consts.tile([120, NCH], F32, tag="amb")
        nc.sync.dma_start(out=amb_sb, in_=amb)
        id_sb = consts.tile([120, 120], BF16, tag="idt")
        nc.sync.dma_start(out=id_sb, in_=idt)
        wo_sb = consts.tile([128, 2, H], BF16, tag="wo")
        nc.sync.dma_start(out=wo_sb, in_=wo.rearrange("(c p) m -> p c m", p=128))

        # expand RoPE tables [128, T] -> [128, T, SP] on-chip (DVE)
        cos_e = consts.tile([128, T, SP], BF16, tag="cos")
        nc.vector.tensor_copy(
            out=cos_e, in_=cosP_sb[:, :, None].broadcast_to([128, T, SP]))
        sin_e = consts.tile([128, T, SP], BF16, tag="sin")
        nc.vector.tensor_copy(
            out=sin_e, in_=sinP_sb[:, :, None].broadcast_to([128, T, SP]))
        cos_sb = cos_e.rearrange("p t s -> p (t s)")
        sin_sb = sin_e.rearrange("p t s -> p (t s)")

        # ---- persistent activations ----
        qT = [persist.tile([128, S], BF16, tag=f"qT{hp}", name=f"qT{hp}")
              for hp in range(2)]
        kT = [persist.tile([128, S], BF16, tag=f"kT{hp}", name=f"kT{hp}")
              for hp in range(2)]
        ctxT = [persist.tile([128, S], BF16, tag=f"ctxT{hp}", name=f"ctxT{hp}")
                for hp in range(2)]
        # v in natural layout per chunk: [key, head, d+1]; col 64 = denom ones
        v_sb = persist.tile([120, NCH, HPC, D + 1], BF16, tag="v", name="v_sb")
        nc.gpsimd.memset(v_sb[:, :, :, D], 1.0)
        # exp ring: 12 pair-slots (6 chunks deep); frame cols [0:40) are the
        # never-written left pad -> zeroed ONCE so AV contraction skips them
        ERING = 12
        exps = persist.tile([120, ERING, 2, MW], BF16, tag="exps",
                            name="exps")
        nc.gpsimd.memset(exps[:, :, :, 0:40], 0.0)
        exp0 = persist.tile([120, 2, 2, 200], BF16, tag="exp0", name="exp0")

        # ---- merged projection + attention schedule ----
        # qk/v units interleave with attention chunk-steps so the PE stream
        # stays dense (HAM clock-gate stays released at 2.4 GHz)
        units = []
        qk_seq = [(w, hp, sc) for sc in range(NSC)
                  for (w, hp) in (("q", 0), ("k", 0), ("q", 1), ("k", 1))]
        for i, u in enumerate(qk_seq):
            units.append(("qk",) + u)
            if i < NCH:
                units.append(("v", i))
        units.append(("v", 16))
        ucur = [0]
        state = {"tokens_ready": 0, "v_emitted": 0}

        def emit_v(vc, ps):
            ckv = _ck(vc)
            rows = slice(120 * vc, 120 * vc + ckv)
            for kc in range(4):
                nc.tensor.matmul(
                    ps[0:ckv, :],
                    lhsT=xt[kc][:, rows],
                    rhs=wv_sb[:, kc, :],
                    start=(kc == 0), stop=(kc == 3),
                )
            vdst = v_sb[0:ckv, vc, :, 0:D]
            vsrc = ps[0:ckv, 0:HG].rearrange("p (h e) -> p h e", e=D)
            if vc % 2 == 0:
                nc.scalar.copy(out=vdst, in_=vsrc)
            else:
                nc.vector.tensor_copy(out=vdst, in_=vsrc)
            state["v_emitted"] += 1

        # PE warm-up: junk matmuls on p128 while the first DMAs land
        wps = utile()
        for r in range(48):
            nc.tensor.matmul(wps[:, 0:128], lhsT=p_sb, rhs=p_sb,
                             start=True, stop=True)
        wdump = work.tile([1, 1], F32, tag="wdump", bufs=1)
        nc.vector.tensor_copy(out=wdump, in_=wps[0:1, 0:1])

        rot_q = []          # pending rotate-half jobs: (w, hp, sc, ps, pre)
        rots_done = {sc: 0 for sc in range(NSC)}

        def emit_rot():
            w, hp, sc, ps, pre = rot_q.pop(0)
            cols = slice(SC * sc, SC * (sc + 1))
            dst = (qT if w == "q" else kT)[hp]
            nc.tensor.matmul(ps, lhsT=p_sb, rhs=pre, start=True, stop=True)
            t1 = work.tile([128, SC], BF16, tag="t1")
            nc.vector.tensor_mul(out=t1, in0=pre, in1=cos_sb[:, cols])
            t2 = work.tile([128, SC], BF16, tag="t2")
            nc.vector.tensor_mul(out=t2, in0=ps, in1=sin_sb[:, cols])
            nc.gpsimd.tensor_add(out=dst[:, cols], in0=t1, in1=t2)
            rots_done[sc] += 1
            if rots_done[sc] == 4:
                state["tokens_ready"] = SC * (sc + 1)

        def emit_unit():
            if ucur[0] >= len(units):
                while rot_q:
                    emit_rot()
                return False
            unit = units[ucur[0]]
            ucur[0] += 1
            if unit[0] == "qk":
                _, w, hp, sc = unit
                cols = slice(SC * sc, SC * (sc + 1))
                w_sb = wq_sb if w == "q" else wk_sb
                ps = utile()[:, 0:SC]
                for kc in range(4):
                    nc.tensor.matmul(
                        ps,
                        lhsT=w_sb[:, kc, 128 * hp:128 * (hp + 1)],
                        rhs=xt[kc][:, cols],
                        start=(kc == 0), stop=(kc == 3),
                    )
                pre = work.tile([128, SC], BF16, tag="pre", bufs=4)
                nc.scalar.copy(out=pre, in_=ps)
                rot_q.append((w, hp, sc, ps, pre))
            else:
                emit_v(unit[1], uvtile()[:, 0:HG])
            if len(rot_q) >= 2:
                emit_rot()
            return True

        # ---- attention ----
        exp_t = {}      # (j, h) -> (exp AP [keys, MW-ish], frame base token)

        def scores_chunk(j, p):
            qlo, qhi = _qwin(j)
            ckj = _ck(j)
            cj = slice(120 * j, 120 * j + ckj)
            fb = 0 if j == 0 else 120 * (j - 1)   # frame base token
            c0 = 0 if j == 0 else 40              # first written frame col
            w = qhi - fb
            ps = u2tile()
            for e in range(2):
                h = 2 * p + e
                hp, hb = h // 2, 64 * (h % 2)
                nc.tensor.matmul(
                    ps[0:ckj, e, c0:w],
                    lhsT=kT[hp][hb:hb + 64, cj],
                    rhs=qT[hp][hb:hb + 64, fb + c0:qhi],
                    start=True, stop=True,
                )
            et = exp0[:, p] if j == 0 else exps[:, (2 * j + p) % ERING]
            nc.scalar.activation(out=et[0:ckj, :, c0:w],
                                 in_=ps[0:ckj, :, c0:w],
                                 func=mybir.ActivationFunctionType.Exp,
                                 scale=0.125,
                                 bias=amb_sb[0:ckj, j:j + 1])
            mk = mf_sb[0:ckj, 0:w] if j == 0 else mi_sb[0:ckj, 0:w - 40]
            nc.vector.tensor_mul(
                out=et[0:ckj, :, c0:w], in0=et[0:ckj, :, c0:w],
                in1=mk[:, None, :].broadcast_to([ckj, 2, w - c0]))
            for e in range(2):
                exp_t[(j, 2 * p + e)] = (et[:, e], fb)

        pend_tr = []

        def pop_tr(n=1):
            for _ in range(n):
                if not pend_tr:
                    return
                i, hp = pend_tr.pop(0)
                cki = _ck(i)
                cs = tr_cs[i]
                csf = cs.rearrange("p h e -> p (h e)")
                ptr = pt.tile([128, 120], BF16, tag="t", name="ptr")
                nc.tensor.transpose(ptr[:, 0:cki],
                                    csf[0:cki, 128 * hp:128 * (hp + 1)],
                                    id_sb[0:cki, 0:cki])
                nc.vector.tensor_copy(
                    out=ctxT[hp][:, 120 * i:120 * i + cki], in_=ptr[:, 0:cki])

        tr_cs = {}

        def av_strip(i):
            # chunk i first: it covers the strip fully (start=True sets
            # has_written; neighbors accumulate on partial row ranges)
            cki = _ck(i)
            chunks = [c for c in (i, i - 1, i + 1) if 0 <= c < NCH]
            psu = pavtile()
            ps = psu[:, 0:HPC * (D + 1)].rearrange("p (h e) -> p h e", e=D + 1)
            for h in range(HPC):
                for n, jc in enumerate(chunks):
                    qlo, qhi = _qwin(jc)
                    lo_g = max(120 * i, qlo)
                    hi_g = min(120 * i + cki, qhi)
                    assert lo_g == 120 * i
                    ev, fb = exp_t[(jc, h)]
                    nc.tensor.matmul(
                        ps[0:hi_g - lo_g, h, :],
                        lhsT=ev[0:_ck(jc), lo_g - fb:hi_g - fb],
                        rhs=v_sb[0:_ck(jc), jc, h, :],
                        start=(n == 0), stop=(n == len(chunks) - 1),
                    )
            # softmax normalize: rcp of denom col, ONE broadcast multiply
            rcp = work.tile([120, HPC], F32, tag="rcp")
            nc.vector.reciprocal(out=rcp[0:cki, :], in_=ps[0:cki, :, D])
            cs = work.tile([120, HPC, D], BF16, tag="cs", bufs=4)
            nc.vector.tensor_mul(
                out=cs[0:cki], in0=ps[0:cki, :, 0:D],
                in1=rcp[0:cki, :, None].broadcast_to([cki, HPC, D]))
            if dbg:
                nc.sync.dma_start(out=cso[i, 0:cki], in_=cs[0:cki])
            tr_cs[i] = cs
            pend_tr.append((i, 0))
            pend_tr.append((i, 1))

        def out_proj(lo, hi):
            for oc in range(4):
                ps = utile()[:, 0:hi - lo]
                for hp in range(2):
                    nc.tensor.matmul(
                        ps,
                        lhsT=wo_sb[:, hp, 128 * oc:128 * (oc + 1)],
                        rhs=ctxT[hp][:, lo:hi],
                        start=(hp == 0), stop=(hp == 1),
                    )
                ost = work.tile([128, SC], F32, tag="ost")
                if oc % 2 == 0:
                    nc.scalar.copy(out=ost[:, 0:hi - lo], in_=ps)
                else:
                    nc.vector.tensor_copy(out=ost[:, 0:hi - lo], in_=ps)
                nc.sync.dma_start(out=outT[128 * oc:128 * (oc + 1), lo:hi],
                                  in_=ost[:, 0:hi - lo])

        for j in range(NCH):
            while (state["tokens_ready"] < min(S, _qwin(j)[1] + 400)
                   or state["v_emitted"] < min(NCH, j + 2)):
                if not emit_unit():
                    break
            scores_chunk(j, 0)
            emit_unit()
            pop_tr()
            scores_chunk(j, 1)
            emit_unit()
            pop_tr()
            if j >= 3:
                av_strip(j - 3)
                emit_unit()
                if j - 3 == 9:
                    out_proj(0, 500)
                    out_proj(500, 1000)
                elif j - 3 == 13:
                    out_proj(1000, 1500)
        while emit_unit():
            pass
        av_strip(NCH - 3)
        pop_tr()
        av_strip(NCH - 2)
        pop_tr(3)
        out_proj(1500, 1800)
        av_strip(NCH - 1)
        pop_tr(4)
        out_proj(1800, 2000)
        if dbg:
            for hp in range(2):
                nc.sync.dma_start(out=qTo[hp], in_=qT[hp])
                nc.sync.dma_start(out=kTo[hp], in_=kT[hp])
                nc.sync.dma_start(out=cxo[hp], in_=ctxT[hp])
            nc.sync.dma_start(out=vo, in_=v_sb)

    nc.finalize()   # Bacc register allocation + DCE before serialization
    return nc


def _get_program():
    if "nc" not in _CACHE:
        _CACHE["nc"] = _build_program()
    return _CACHE["nc"]


def _host_prep(x, attn_mask, timestamps, Wq, Wk, Wv, Wo):
    """Build the 8 per-core input maps."""
    bf16 = ml_dtypes.bfloat16

    def to_tm(a):
        # [B, S, ...] space-major -> time-major (u = t*SP + sp)
        return (a.reshape(B, SP, T, *a.shape[2:])
                 .swapaxes(1, 2)
                 .reshape(B, S, *a.shape[2:]))

    x_tm = to_tm(np.ascontiguousarray(x))
    amask_tm = to_tm(np.ascontiguousarray(attn_mask)).astype(np.float32)

    # RoPE tables (per time patch; expanded to tokens on-chip)
    inv_freq = 1.0 / (ROPE_BASE ** (np.arange(0, D, 2, dtype=np.float32) / D))
    tt = np.arange(T, dtype=np.float32)
    freqs = tt[:, None] * inv_freq[None, :]
    emb = np.concatenate([freqs, freqs], axis=-1)      # [T, D]
    cos_t = np.cos(emb).astype(np.float32).T           # [64, T]
    sin_t = np.sin(emb).astype(np.float32).T
    cosP = np.vstack([cos_t, cos_t])                   # [128, T]
    sinP = np.vstack([sin_t, sin_t])

    # rotation matrix (sign-carrying rotate-half), block-diag per head pair
    p = np.zeros((128, 128), np.float32)
    for blk in (0, 64):
        for d in range(32):
            p[blk + d + 32, blk + d] = -1.0
            p[blk + d, blk + d + 32] = 1.0

    # band masks: interior chunk j has frame at patch 6(j-1); key k (patch
    # r=k//20) sees query rel-patch pc iff pc - r in [2, 10]; the exp tile
    # stores cols [40:w] so m_int covers frame cols 40..320
    rr = np.arange(120)[:, None] // SP
    pc_i = (np.arange(MW - 40)[None, :] + 40) // SP
    m_int = ((pc_i - rr >= 2) & (pc_i - rr <= 10)).astype(np.float32)
    pc_f = np.arange(200)[None, :] // SP
    m_first = ((pc_f - rr >= -4) & (pc_f - rr <= 4)).astype(np.float32)

    # attn_mask bias per (key-in-chunk, chunk)
    def amb_of(am_b):
        out = np.zeros((120, NCH), np.float32)
        for j in range(NCH):
            ckj = _ck(j)
            out[0:ckj, j] = NEGB * (1.0 - am_b[120 * j:120 * j + ckj])
        return out

    in_maps = []
    for c in range(N_CORES):
        b, g = c // 2, c % 2
        hcols = slice(HG * g, HG * (g + 1))
        in_maps.append({
            "xT": np.ascontiguousarray(x_tm[b].T).astype(bf16),
            "wq": np.ascontiguousarray(Wq[:, hcols]).astype(bf16),
            "wk": np.ascontiguousarray(Wk[:, hcols]).astype(bf16),
            "wv": np.ascontiguousarray(Wv[:, hcols]).astype(bf16),
            "wo": np.ascontiguousarray(Wo[hcols, :]).astype(bf16),
            "cosP": cosP.astype(bf16),
            "sinP": sinP.astype(bf16),
            "p128": p.astype(bf16),
            "idt": np.eye(120, dtype=np.float32).astype(bf16),
            "m_first": m_first.astype(bf16),
            "m_int": m_int.astype(bf16),
            "amb": amb_of(amask_tm[b]),
        })
    return in_maps


def kernel(x, attn_mask, timestamps, Wq, bq, Wk, bk, Wv, bv, Wo, bo,
           **_ignored):
    x = np.asarray(x, np.float32)
    attn_mask = np.asarray(attn_mask)
    timestamps = np.asarray(timestamps)
    Wq, Wk, Wv, Wo = (np.asarray(a, np.float32) for a in (Wq, Wk, Wv, Wo))
    bq, bk, bv, bo = (np.asarray(a, np.float32) for a in (bq, bk, bv, bo))
    assert not (np.any(bq) or np.any(bk) or np.any(bv)), \
        "nonzero qkv biases not supported"

    nc = _get_program()
    in_maps = _host_prep(x, attn_mask, timestamps, Wq, Wk, Wv, Wo)

    res = bass_utils.run_bass_kernel_spmd(nc, in_maps,
                                          core_ids=list(range(N_CORES)))
    _CACHE["last_results"] = res

    out = np.empty((B, S, H), np.float32)
    for b in range(B):
        o = res.results[2 * b]["outT"] + res.results[2 * b + 1]["outT"]
        o_tm = o.T + bo[None, :]                        # [2000, 512]
        out[b] = (o_tm.reshape(T, SP, H)
                      .swapaxes(0, 1)
                      .reshape(S, H))
    return out
